# revision 17
# baseline (speedup 1.0000x reference)
"""STFT magnitude spectrogram kernel for Trainium2 (8 NeuronCores).

Computes, for x (64, 160000):
  out[b, k, t] = |sum_n w[n] * x[b, 256*t + n] * exp(-2i*pi*k*n/1024)|
with w the normalized Hann window from the reference (wl=1024, pow=1).
Data-parallel over batch: 8 rows per core.

Device algorithm per core (8 batch rows), "half-sample fold":
  Using the window symmetry w[512+j] = w[511-j] and writing m = n-511.5,
    |X[k]| = sqrt(c[k]^2 + s[k]^2)
    c[k] = sum_j w~[j] cos(2pi k (j+.5)/1024) * u[j],  u = x[512+j+.]+x[511-j+.]
    s[k] = sum_j w~[j] sin(2pi k (j+.5)/1024) * v[j],  v = x[512+j+.]-x[511-j+.]
  which HALVES the DFT contraction (512 instead of 1024) with zero padding
  waste: cos rows cover k=0..511 (slot 0 zeroed), sin rows cover k=1..511
  plus the Nyquist k=512 pattern w~[j]*(-1)^j in slot 0.  Row k=0 is done
  on the host (single dot product per frame).

  1. SWDGE cast-DMA x into SBUF natural layout as bf16.
  2. Wide XBAR DMA-transposes build sample-on-partition streams
     S_h[p,q] = x[256q+128h+p] (one instruction per (row-pair, j)).
  3. Reversed streams R_h[p,q] = S_h[127-p,q] via PE matmul with an
     anti-identity (prologue PSUM pool), copied to SBUF bf16 by ACT/DVE.
  4. DVE folds (bf16, 2x mode): u, v tiles per row.
  5. bf16 matmuls vs window-folded DFT weights, accumulated over 4
     chunks of 128 in PSUM: c and s tiles per (ktile, t-half).
  6. Magnitude: square of s (mostly ACT, some DVE), custom DVE op
     sq(c)+s2 (max one PSUM operand per instruction), ACT sqrt -> bf16.
     Output DMA'd as bf16 in slot layout; host upcasts and reshuffles.
"""

import sys

sys.path.insert(0, "/opt/trn_rl_repo")

import numpy as np

N = 1024
STRIDE = 256
B = 64
L = 160000
T = 622          # frames
F = 513          # rfft bins
NCORES = 8
BPC = B // NCORES  # batch rows per core
NCH = 4            # contraction chunks of 128 (over j = 0..511)
NMT = 4            # output k-tiles of 128 (slots 0..511)
WSPLIT = 311       # t-half width (622 = 2*311; PSUM bank limit is 512 f32)
LMAIN = 4 * 128 * 256  # 131072 samples in the "main" part of a row
PTAIL = (L - LMAIN) // 256  # 113 partitions in the tail part

_prog_cache = {}


def _patch_fast_compile():
    """Disable the BIR simulator inside walrus codegen: it is only a
    verification aid and costs ~50 min on this kernel (vs ~3 min off)."""
    import concourse.bass_utils as bu

    if getattr(bu, "_fast_compile_patched", False):
        return
    from pathlib import Path

    from concourse.aot_env import aot_getenv

    def bir_verify_and_optimise(
        tmpdir, inp="bir.json", outp="file.neff", arch=None, *, dve_root=None
    ):
        cmd = [
            bu.get_walrus_driver(),
            "--pass",
            ",".join(
                [
                    "birverifier",
                    "runtime_memory_reservation",
                    "lower_act",
                    "lower_dve",
                    "lower_ap_offset",
                    "codegen",
                    "neff_packager",
                ]
            ),
            "-i", inp,
            "--neff-output-filename", outp,
            "--enable-birsim=false",
            "--mem-mode=physical",
            "--policy=0",
            "--enable-ldw-opt=false",
            "--assign-static-dmas-to-sp=false",
            f"--dram-page-size={aot_getenv('NEURON_SCRATCHPAD_PAGE_SIZE', '256')}",
            "--enable-neff-debug-info=true",
            "--jobs", "8",
            *bu.get_walrus_args(
                bu.get_bir_arch(tmpdir, inp) if arch is None else arch,
                tmpdir,
                dve_root=dve_root,
            ),
        ]
        result = bu.run_command(cmd, cwd=tmpdir)
        if result is not None:
            (Path(tmpdir) / "log.txt").write_text(result.stdout)
        return f"{tmpdir}/{outp}"

    bu.bir_verify_and_optimise = bir_verify_and_optimise
    bu._fast_compile_patched = True


def _register_dve_ops():
    """Register the custom DVE ops used by the magnitude stage."""
    from concourse import dve_ops
    from concourse.dve_spec import Spec, Src0, Src1, lower, sq
    from concourse.dve_uop import DveOpSpec

    def reg(name, spec):
        for op in dve_ops.OPS:
            if op.name == name:
                return op
        row = dve_ops._CUSTOM_DVE_ROW_BASE + len(dve_ops.OPS)
        tmp = DveOpSpec(
            name=name, opcode=row, uops=lower(spec, ver="v3"),
            rd1_en=dve_ops.has_src1(spec),
        )
        op = dve_ops.DveOp(name, spec, subdim=False, uops_sha={"v3": tmp.sha("v3")})
        dve_ops.OPS.append(op)
        dve_ops.CUSTOM_DVE_SPECS[name] = spec
        dve_ops._SUB_OPCODE_FOR_NAME[name] = row
        return op

    mag2b = reg(
        "MAG2B_ANT",
        Spec(
            body=sq(Src0) + Src1,
            reference=lambda in0, in1, s0, s1, imm2: (
                in0.astype(np.float32) ** 2 + in1.astype(np.float32)
            ),
        ),
    )
    sq1 = reg(
        "SQ1_ANT",
        Spec(
            body=sq(Src0),
            reference=lambda in0, in1, s0, s1, imm2: in0.astype(np.float32) ** 2,
        ),
    )
    return mag2b, sq1


def _build_program():
    _patch_fast_compile()
    import concourse.mybir as mybir
    import concourse.tile as tile
    from concourse import bacc
    from concourse.ap import AP

    MAG2B, SQ1 = _register_dve_ops()

    f32 = mybir.dt.float32
    bf16 = mybir.dt.bfloat16
    Sqrt = mybir.ActivationFunctionType.Sqrt

    nc = bacc.Bacc("TRN2", target_bir_lowering=False, enable_partition_id=False)

    xs = nc.dram_tensor("xs", [BPC, L], f32, kind="ExternalInput")
    cw = nc.dram_tensor("cw", [128, NCH, 512], bf16, kind="ExternalInput")
    sw = nc.dram_tensor("sw", [128, NCH, 512], bf16, kind="ExternalInput")
    jm = nc.dram_tensor("jm", [128, 128], bf16, kind="ExternalInput")
    # slot layout: out[b, m, p, t] = |X[k]| with k = 128 m + p, except
    # slot (0, 0) which is the Nyquist row k=512 (k=0 is done on host)
    out = nc.dram_tensor("out", [BPC, NMT, 128, T], bf16, kind="ExternalOutput")

    with tile.TileContext(nc) as tc:
        with (
            tc.tile_pool(name="const", bufs=1) as const_pool,
            tc.tile_pool(name="stream", bufs=4) as stream_pool,
            tc.tile_pool(name="rstr", bufs=BPC) as rstr_pool,
            tc.tile_pool(name="pmm", bufs=1, space="PSUM") as pmm_pool,
            tc.tile_pool(name="xn", bufs=4) as xn_pool,
            tc.tile_pool(name="uv", bufs=2) as uv_pool,
            tc.tile_pool(name="sq", bufs=3) as sq_pool,
            tc.tile_pool(name="outsb", bufs=3) as out_pool,
        ):
            cw_sb = const_pool.tile([128, NCH, 512], bf16)
            sw_sb = const_pool.tile([128, NCH, 512], bf16)
            jm_sb = const_pool.tile([128, 128], bf16)
            nc.sync.dma_start(jm_sb[:], jm[:, :])
            for c in range(NCH):
                nc.sync.dma_start(cw_sb[:, c, :], cw[:, c, :])
                nc.sync.dma_start(sw_sb[:, c, :], sw[:, c, :])

            # --- streams per row-pair: S[p, rr, h, q] = x[b0+rr, 256q+128h+p]
            # All casts are emitted before all transposes so the 16 DMA
            # queue engines run concurrently (in-order engines + buffer
            # reuse would otherwise chain pair k's casts behind pair k-1's
            # transposes).
            spair, xqs, xts = [], [], []
            PART = [2 * 2 * 640, 128]
            JBLK = 32768
            for pi in range(4):
                b0 = 2 * pi
                st = stream_pool.tile([128, 2, 2, 640], bf16, tag="spair",
                                      name=f"spair{pi}")
                xq = xn_pool.tile([128, 4, 2, 256], bf16, tag="xq",
                                  name=f"xq{pi}")
                xt = xn_pool.tile([128, 2, 256], bf16, tag="xt",
                                  name=f"xt{pi}")
                # one cast-DMA per (row, j-block): ~128 descriptors each, so
                # the SWDGE queue engines work in parallel (a whole-row cast
                # is 625 serialized small packets = ~29 us latency)
                for rr in range(2):
                    for j in range(4):
                        nc.gpsimd.dma_start(
                            xq[:, j, rr, :],
                            xs[b0 + rr, JBLK * j : JBLK * (j + 1)].rearrange(
                                "(p r) -> p r", p=128, r=256
                            ),
                        )
                    nc.gpsimd.dma_start(
                        xt[0:PTAIL, rr, :],
                        xs[b0 + rr, LMAIN:L].rearrange(
                            "(p r) -> p r", p=PTAIL, r=256
                        ),
                    )
                spair.append(st)
                xqs.append(xq)
                xts.append(xt)
            for pi in range(4):
                # wide transposes: in [128, 512] -> 4 pages of 128 cols,
                # page g -> (row rr=g>>1, stream h=g&1) at q-block j
                sv = spair[pi][:]
                for j in range(4):
                    dst = AP(sv.tensor, sv.offset + 128 * j,
                             [PART, [640, 4], [1, 128]])
                    nc.sync.dma_start(dst, xqs[pi][:, j, :, :], transpose=True)
                dst = AP(sv.tensor, sv.offset + 512,
                         [PART, [640, 4], [1, 128]])
                nc.sync.dma_start(dst, xts[pi][:], transpose=True)

            def r_gen(r):
                # reversed streams: R_h[p, q] = S_h[127-p, q]  via  J^T @ S
                sv = spair[r // 2][:]
                rt = rstr_pool.tile([128, 2, 640], bf16, tag="rt",
                                    name=f"rt{r}")
                for h in range(2):
                    base = (r & 1) * 1280 + h * 640
                    s512 = AP(sv.tensor, sv.offset + base, [PART, [1, 512]])
                    s113 = AP(sv.tensor, sv.offset + base + 512,
                              [PART, [1, 128]])
                    rp512 = pmm_pool.tile([128, 512], f32, tag="rp512",
                                          name="rp512")
                    rp128 = pmm_pool.tile([128, 128], f32, tag="rp128",
                                          name="rp128")
                    nc.tensor.matmul(rp512[:], jm_sb[:], s512,
                                     start=True, stop=True)
                    nc.tensor.matmul(rp128[:], jm_sb[:], s113,
                                     start=True, stop=True)
                    nc.scalar.copy(rt[:, h, 0:512], rp512[:])
                    nc.vector.tensor_copy(rt[:, h, 512:640], rp128[:])
                return rt

            rtiles = {0: r_gen(0)}
            for r in range(BPC):
                sv = spair[r // 2][:]
                rv = rtiles.pop(r)[:]
                RPART = [2 * 640, 128]
                fbase = (r & 1) * 1280
                # folds: u chunks {c,c+2} = S_h[{2,3}+t] +/- R_h'[{1,0}+t]
                uvt = {}
                for name, h, op in (
                    ("uA", 0, "add"), ("uB", 1, "add"),
                    ("vA", 0, "sub"), ("vB", 1, "sub"),
                ):
                    tl = uv_pool.tile([128, 2, T], bf16, tag=name, name=name)
                    fwd = AP(sv.tensor, sv.offset + fbase + 640 * h + 2,
                             [PART, [1, 2], [1, T]])
                    bwd = AP(rv.tensor, rv.offset + 640 * (1 - h) + 1,
                             [RPART, [-1, 2], [1, T]])
                    if op == "add":
                        nc.vector.tensor_add(tl[:], fwd, bwd)
                    else:
                        nc.vector.tensor_sub(tl[:], fwd, bwd)
                    uvt[name] = tl

                for m in range(NMT):
                    if m == 2 and r + 1 < BPC:
                        # generate the next row's reversed streams mid-row so
                        # its fold chain has a half-row of lead time
                        rtiles[r + 1] = r_gen(r + 1)
                    pc = [pmm_pool.tile([128, WSPLIT], f32, tag=f"pc{ti}",
                                        name=f"pc{ti}", bufs=2)
                          for ti in range(2)]
                    ps = [pmm_pool.tile([128, WSPLIT], f32, tag=f"ps{ti}",
                                        name=f"ps{ti}") for ti in range(2)]
                    for wsb, uv0, uv1, ptile in (
                        (cw_sb, uvt["uA"], uvt["uB"], pc),
                        (sw_sb, uvt["vA"], uvt["vB"], ps),
                    ):
                        for c in range(NCH):
                            w = wsb[:, c, 128 * m : 128 * m + 128]
                            rhs_t = (uv0, uv1)[c & 1]
                            pg = c >> 1
                            kw = dict(start=(c == 0), stop=(c == NCH - 1))
                            for ti in range(2):
                                t0 = ti * WSPLIT
                                nc.tensor.matmul(
                                    ptile[ti][:], w,
                                    rhs_t[:, pg, t0 : t0 + WSPLIT], **kw
                                )
                    sqs = sq_pool.tile([128, T], bf16, tag="sqs")
                    for ti in range(2):
                        sl = sqs[:, ti * WSPLIT : (ti + 1) * WSPLIT]
                        if m == 3 and r >= 2:
                            nc.vector._custom_dve(SQ1, out=sl, in0=ps[ti][:])
                        else:
                            nc.scalar.square(sl, ps[ti][:])
                    mag = sq_pool.tile([128, T], bf16, tag="mag")
                    for ti in range(2):
                        sl = slice(ti * WSPLIT, (ti + 1) * WSPLIT)
                        nc.vector._custom_dve(
                            MAG2B, out=mag[:, sl], in0=pc[ti][:],
                            in1=sqs[:, sl],
                        )
                    osb = out_pool.tile([128, T], bf16, tag="osb")
                    nc.scalar.activation(osb[:], mag[:], Sqrt)
                    nc.gpsimd.dma_start(out[r, m, :, :], osb[:])

    nc.compile()
    return nc


def _host_params(win_length, strides, win_pow):
    """Reproduce the reference's parameter transforms on the host."""
    wl = float(np.clip(np.asarray(win_length, np.float64)[0], N / 20.0, float(N)))
    st = float(np.clip(np.asarray(strides, np.float64)[0], 0.0, float(N)))

    es = np.full((T,), st, np.float64)
    frames = np.concatenate([[0.0], np.cumsum(es[1:])])
    idx_floor = np.floor(frames)
    idx_frac = frames - idx_floor

    if not (np.all(idx_frac == 0.0) and np.all(idx_floor == STRIDE * np.arange(T))):
        raise NotImplementedError(
            "kernel fast path requires integer frame stride of 256"
        )

    base = np.arange(N, dtype=np.float64)
    tap = 0.5 - 0.5 * np.cos(2.0 * np.pi * (base + (wl - N + 1) / 2.0) / wl)
    mask = (base >= np.ceil((N - 1 + wl) / 2.0)) | (base <= np.floor((N - 1 - wl) / 2.0))
    tap[mask] = 0.0
    tap = tap / tap.sum()
    tap = tap ** float(np.asarray(win_pow, np.float64)[0])

    if not np.allclose(tap[512:], tap[511::-1], rtol=1e-12, atol=1e-18):
        raise NotImplementedError("kernel fast path requires a symmetric window")
    return tap


def _make_weights(tap):
    """Window-folded half-sample DFT matrices, bf16, [128, NCH, 512]."""
    import ml_dtypes

    wt = tap[512:]  # w~[j] = tap[512+j], j = 0..511
    j = np.arange(512, dtype=np.float64)
    k = np.arange(512, dtype=np.float64)
    ang = 2.0 * np.pi * np.outer(j + 0.5, k) / float(N)
    C = wt[:, None] * np.cos(ang)
    S = wt[:, None] * np.sin(ang)
    C[:, 0] = 0.0                      # slot 0 of c is unused (k=0 on host)
    S[:, 0] = wt * ((-1.0) ** j)       # slot 0 of s is the Nyquist k=512 row

    def pack(M):
        # [512 j, 512 k] -> [128 p, NCH c, 512 k] with j = 128 c + p
        return np.ascontiguousarray(
            M.reshape(NCH, 128, 512).transpose(1, 0, 2).astype(ml_dtypes.bfloat16)
        )

    return pack(C), pack(S)


def _make_in_maps(x, tap):
    import ml_dtypes

    cwb, swb = _make_weights(tap)
    jmb = np.ascontiguousarray(
        np.eye(128, dtype=np.float32)[::-1].astype(ml_dtypes.bfloat16)
    )
    return [
        {
            "xs": np.ascontiguousarray(x[c * BPC : (c + 1) * BPC]),
            "cw": cwb,
            "sw": swb,
            "jm": jmb,
        }
        for c in range(NCORES)
    ]


def kernel(x, win_length, strides, win_pow):
    from concourse.bass_utils import run_bass_kernel_spmd

    x = np.ascontiguousarray(np.asarray(x, dtype=np.float32))
    assert x.shape == (B, L)

    tap = _host_params(win_length, strides, win_pow)

    if "nc" not in _prog_cache:
        _prog_cache["nc"] = _build_program()
    nc = _prog_cache["nc"]

    in_maps = _make_in_maps(x, tap)
    res = run_bass_kernel_spmd(nc, in_maps, core_ids=list(range(NCORES)))
    outp = np.empty((B, F, T), dtype=np.float32)
    for c in range(NCORES):
        o = np.asarray(res.results[c]["out"]).astype(np.float32)
        o = o.reshape(BPC, NMT * 128, T)
        rows = slice(c * BPC, (c + 1) * BPC)
        outp[rows, 1:512] = o[:, 1:512]
        outp[rows, 512] = o[:, 0]       # slot (0,0) carries the Nyquist row

    # k=0 (DC) row on host: X[0] = sum_n w[n] x[., 256t + n]
    wn = tap.astype(np.float32)
    frames_v = np.lib.stride_tricks.as_strided(
        x,
        shape=(B, T, N),
        strides=(x.strides[0], STRIDE * x.itemsize, x.itemsize),
    )
    outp[:, 0, :] = np.abs(frames_v @ wn)
    return outp


# revision 19
# speedup vs baseline: 1.3543x; 1.3543x over previous
"""STFT magnitude spectrogram kernel for Trainium2 (8 NeuronCores).

Computes, for x (64, 160000):
  out[b, k, t] = |sum_n w[n] * x[b, 256*t + n] * exp(-2i*pi*k*n/1024)|
with w the normalized Hann window from the reference (wl=1024, pow=1).
Data-parallel over batch: 8 rows per core.

Device algorithm per core (8 batch rows), "half-sample fold":
  Using the window symmetry w[512+j] = w[511-j] and writing m = n-511.5,
    |X[k]| = sqrt(c[k]^2 + s[k]^2)
    c[k] = sum_j w~[j] cos(2pi k (j+.5)/1024) * u[j],  u = x[512+j+.]+x[511-j+.]
    s[k] = sum_j w~[j] sin(2pi k (j+.5)/1024) * v[j],  v = x[512+j+.]-x[511-j+.]
  which HALVES the DFT contraction (512 instead of 1024) with zero padding
  waste: cos rows cover k=0..511 (slot 0 zeroed), sin rows cover k=1..511
  plus the Nyquist k=512 pattern w~[j]*(-1)^j in slot 0.  Row k=0 is done
  on the host (single dot product per frame).

  1. SWDGE cast-DMA x into SBUF natural layout as bf16.
  2. Wide XBAR DMA-transposes build sample-on-partition streams
     S_h[p,q] = x[256q+128h+p] (one instruction per (row-pair, j)).
  3. Reversed streams R_h[p,q] = S_h[127-p,q] via PE matmul with an
     anti-identity (prologue PSUM pool), copied to SBUF bf16 by ACT/DVE.
  4. DVE folds (bf16, 2x mode): u, v tiles per row.
  5. bf16 matmuls vs window-folded DFT weights, accumulated over 4
     chunks of 128 in PSUM: c and s tiles per (ktile, t-half).
  6. Magnitude: square of s (mostly ACT, some DVE), custom DVE op
     sq(c)+s2 (max one PSUM operand per instruction), ACT sqrt -> bf16.
     Output DMA'd as bf16 in slot layout; host upcasts and reshuffles.
"""

import sys

sys.path.insert(0, "/opt/trn_rl_repo")

import numpy as np

N = 1024
STRIDE = 256
B = 64
L = 160000
T = 622          # frames
F = 513          # rfft bins
NCORES = 8
BPC = B // NCORES  # batch rows per core
NCH = 4            # contraction chunks of 128 (over j = 0..511)
NMT = 4            # output k-tiles of 128 (slots 0..511)
WSPLIT = 311       # t-half width (622 = 2*311; PSUM bank limit is 512 f32)
LMAIN = 4 * 128 * 256  # 131072 samples in the "main" part of a row
PTAIL = (L - LMAIN) // 256  # 113 partitions in the tail part

_prog_cache = {}


def _patch_fast_compile():
    """Disable the BIR simulator inside walrus codegen: it is only a
    verification aid and costs ~50 min on this kernel (vs ~3 min off)."""
    import concourse.bass_utils as bu

    if getattr(bu, "_fast_compile_patched", False):
        return
    from pathlib import Path

    from concourse.aot_env import aot_getenv

    def bir_verify_and_optimise(
        tmpdir, inp="bir.json", outp="file.neff", arch=None, *, dve_root=None
    ):
        cmd = [
            bu.get_walrus_driver(),
            "--pass",
            ",".join(
                [
                    "birverifier",
                    "runtime_memory_reservation",
                    "lower_act",
                    "lower_dve",
                    "lower_ap_offset",
                    "codegen",
                    "neff_packager",
                ]
            ),
            "-i", inp,
            "--neff-output-filename", outp,
            "--enable-birsim=false",
            "--mem-mode=physical",
            "--policy=0",
            "--enable-ldw-opt=false",
            "--assign-static-dmas-to-sp=false",
            f"--dram-page-size={aot_getenv('NEURON_SCRATCHPAD_PAGE_SIZE', '256')}",
            "--enable-neff-debug-info=true",
            "--jobs", "8",
            *bu.get_walrus_args(
                bu.get_bir_arch(tmpdir, inp) if arch is None else arch,
                tmpdir,
                dve_root=dve_root,
            ),
        ]
        result = bu.run_command(cmd, cwd=tmpdir)
        if result is not None:
            (Path(tmpdir) / "log.txt").write_text(result.stdout)
        return f"{tmpdir}/{outp}"

    bu.bir_verify_and_optimise = bir_verify_and_optimise
    bu._fast_compile_patched = True


def _register_dve_ops():
    """Register the custom DVE ops used by the magnitude stage."""
    from concourse import dve_ops
    from concourse.dve_spec import Spec, Src0, Src1, lower, sq
    from concourse.dve_uop import DveOpSpec

    def reg(name, spec):
        for op in dve_ops.OPS:
            if op.name == name:
                return op
        row = dve_ops._CUSTOM_DVE_ROW_BASE + len(dve_ops.OPS)
        tmp = DveOpSpec(
            name=name, opcode=row, uops=lower(spec, ver="v3"),
            rd1_en=dve_ops.has_src1(spec),
        )
        op = dve_ops.DveOp(name, spec, subdim=False, uops_sha={"v3": tmp.sha("v3")})
        dve_ops.OPS.append(op)
        dve_ops.CUSTOM_DVE_SPECS[name] = spec
        dve_ops._SUB_OPCODE_FOR_NAME[name] = row
        return op

    mag2b = reg(
        "MAG2B_ANT",
        Spec(
            body=sq(Src0) + Src1,
            reference=lambda in0, in1, s0, s1, imm2: (
                in0.astype(np.float32) ** 2 + in1.astype(np.float32)
            ),
        ),
    )
    sq1 = reg(
        "SQ1_ANT",
        Spec(
            body=sq(Src0),
            reference=lambda in0, in1, s0, s1, imm2: in0.astype(np.float32) ** 2,
        ),
    )
    return mag2b, sq1


def _build_program():
    _patch_fast_compile()
    import concourse.mybir as mybir
    import concourse.tile as tile
    from concourse import bacc
    from concourse.ap import AP

    MAG2B, SQ1 = _register_dve_ops()

    f32 = mybir.dt.float32
    bf16 = mybir.dt.bfloat16
    Sqrt = mybir.ActivationFunctionType.Sqrt

    nc = bacc.Bacc("TRN2", target_bir_lowering=False, enable_partition_id=False)

    xs = nc.dram_tensor("xs", [BPC, L], f32, kind="ExternalInput")
    cw = nc.dram_tensor("cw", [128, NCH, 512], bf16, kind="ExternalInput")
    sw = nc.dram_tensor("sw", [128, NCH, 512], bf16, kind="ExternalInput")
    jm = nc.dram_tensor("jm", [128, 128], bf16, kind="ExternalInput")
    # slot layout: out[b, m, p, t] = |X[k]| with k = 128 m + p, except
    # slot (0, 0) which is the Nyquist row k=512 (k=0 is done on host)
    out = nc.dram_tensor("out", [BPC, NMT, 128, T], bf16, kind="ExternalOutput")

    with tile.TileContext(nc) as tc:
        with (
            tc.tile_pool(name="const", bufs=1) as const_pool,
            tc.tile_pool(name="stream", bufs=4) as stream_pool,
            tc.tile_pool(name="rstr", bufs=BPC) as rstr_pool,
            tc.tile_pool(name="pmm", bufs=1, space="PSUM") as pmm_pool,
            tc.tile_pool(name="xn", bufs=4) as xn_pool,
            tc.tile_pool(name="uv", bufs=2) as uv_pool,
            tc.tile_pool(name="sq", bufs=3) as sq_pool,
            tc.tile_pool(name="outsb", bufs=3) as out_pool,
        ):
            cw_sb = const_pool.tile([128, NCH, 512], bf16)
            sw_sb = const_pool.tile([128, NCH, 512], bf16)
            jm_sb = const_pool.tile([128, 128], bf16)
            nc.sync.dma_start(jm_sb[:], jm[:, :])
            for c in range(NCH):
                nc.sync.dma_start(cw_sb[:, c, :], cw[:, c, :])
                nc.sync.dma_start(sw_sb[:, c, :], sw[:, c, :])

            # --- streams per row-pair: S[p, rr, h, q] = x[b0+rr, 256q+128h+p]
            # All casts are emitted before all transposes so the 16 DMA
            # queue engines run concurrently (in-order engines + buffer
            # reuse would otherwise chain pair k's casts behind pair k-1's
            # transposes).
            spair, xqs, xts = [], [], []
            PART = [2 * 2 * 640, 128]
            JBLK = 32768
            for pi in range(4):
                b0 = 2 * pi
                st = stream_pool.tile([128, 2, 2, 640], bf16, tag="spair",
                                      name=f"spair{pi}")
                xqf = xn_pool.tile([128, 4, 2, 256], f32, tag="xqf",
                                   name=f"xqf{pi}")
                xtf = xn_pool.tile([128, 2, 256], f32, tag="xtf",
                                   name=f"xtf{pi}")
                xq = xn_pool.tile([128, 4, 2, 256], bf16, tag="xq",
                                  name=f"xq{pi}")
                xt = xn_pool.tile([128, 2, 256], bf16, tag="xt",
                                  name=f"xt{pi}")
                # f32 loads on HWDGE (hardware descriptor processing; the
                # SWDGE cast path tops out at ~19 B/ns aggregate), one DMA
                # per (row, j-block) so the queue engines run in parallel
                for rr in range(2):
                    for j in range(4):
                        nc.sync.dma_start(
                            xqf[:, j, rr, :],
                            xs[b0 + rr, JBLK * j : JBLK * (j + 1)].rearrange(
                                "(p r) -> p r", p=128, r=256
                            ),
                        )
                    nc.sync.dma_start(
                        xtf[0:PTAIL, rr, :],
                        xs[b0 + rr, LMAIN:L].rearrange(
                            "(p r) -> p r", p=PTAIL, r=256
                        ),
                    )
                # f32 -> bf16 on the (otherwise idle) Pool engine
                nc.gpsimd.tensor_copy(xq[:], xqf[:])
                nc.gpsimd.tensor_copy(xt[:], xtf[:])
                spair.append(st)
                xqs.append(xq)
                xts.append(xt)
            for pi in range(4):
                # wide transposes: in [128, 512] -> 4 pages of 128 cols,
                # page g -> (row rr=g>>1, stream h=g&1) at q-block j
                sv = spair[pi][:]
                for j in range(4):
                    dst = AP(sv.tensor, sv.offset + 128 * j,
                             [PART, [640, 4], [1, 128]])
                    nc.sync.dma_start(dst, xqs[pi][:, j, :, :], transpose=True)
                dst = AP(sv.tensor, sv.offset + 512,
                         [PART, [640, 4], [1, 128]])
                nc.sync.dma_start(dst, xts[pi][:], transpose=True)

            def r_gen(r):
                # reversed streams: R_h[p, q] = S_h[127-p, q]  via  J^T @ S
                sv = spair[r // 2][:]
                rt = rstr_pool.tile([128, 2, 640], bf16, tag="rt",
                                    name=f"rt{r}")
                for h in range(2):
                    base = (r & 1) * 1280 + h * 640
                    s512 = AP(sv.tensor, sv.offset + base, [PART, [1, 512]])
                    s113 = AP(sv.tensor, sv.offset + base + 512,
                              [PART, [1, 128]])
                    rp512 = pmm_pool.tile([128, 512], f32, tag="rp512",
                                          name="rp512")
                    rp128 = pmm_pool.tile([128, 128], f32, tag="rp128",
                                          name="rp128")
                    nc.tensor.matmul(rp512[:], jm_sb[:], s512,
                                     start=True, stop=True)
                    nc.tensor.matmul(rp128[:], jm_sb[:], s113,
                                     start=True, stop=True)
                    nc.scalar.copy(rt[:, h, 0:512], rp512[:])
                    nc.vector.tensor_copy(rt[:, h, 512:640], rp128[:])
                return rt

            rtiles = {0: r_gen(0)}
            for r in range(BPC):
                sv = spair[r // 2][:]
                rv = rtiles.pop(r)[:]
                RPART = [2 * 640, 128]
                fbase = (r & 1) * 1280
                # folds: u chunks {c,c+2} = S_h[{2,3}+t] +/- R_h'[{1,0}+t]
                uvt = {}
                for name, h, op in (
                    ("uA", 0, "add"), ("uB", 1, "add"),
                    ("vA", 0, "sub"), ("vB", 1, "sub"),
                ):
                    tl = uv_pool.tile([128, 2, T], bf16, tag=name, name=name)
                    fwd = AP(sv.tensor, sv.offset + fbase + 640 * h + 2,
                             [PART, [1, 2], [1, T]])
                    bwd = AP(rv.tensor, rv.offset + 640 * (1 - h) + 1,
                             [RPART, [-1, 2], [1, T]])
                    if op == "add":
                        nc.vector.tensor_add(tl[:], fwd, bwd)
                    else:
                        nc.vector.tensor_sub(tl[:], fwd, bwd)
                    uvt[name] = tl

                for m in range(NMT):
                    if m == 2 and r + 1 < BPC:
                        # generate the next row's reversed streams mid-row so
                        # its fold chain has a half-row of lead time
                        rtiles[r + 1] = r_gen(r + 1)
                    pc = [pmm_pool.tile([128, WSPLIT], f32, tag=f"pc{ti}",
                                        name=f"pc{ti}", bufs=2)
                          for ti in range(2)]
                    ps = [pmm_pool.tile([128, WSPLIT], f32, tag=f"ps{ti}",
                                        name=f"ps{ti}") for ti in range(2)]
                    for wsb, uv0, uv1, ptile in (
                        (cw_sb, uvt["uA"], uvt["uB"], pc),
                        (sw_sb, uvt["vA"], uvt["vB"], ps),
                    ):
                        for c in range(NCH):
                            w = wsb[:, c, 128 * m : 128 * m + 128]
                            rhs_t = (uv0, uv1)[c & 1]
                            pg = c >> 1
                            kw = dict(start=(c == 0), stop=(c == NCH - 1))
                            for ti in range(2):
                                t0 = ti * WSPLIT
                                nc.tensor.matmul(
                                    ptile[ti][:], w,
                                    rhs_t[:, pg, t0 : t0 + WSPLIT], **kw
                                )
                    sqs = sq_pool.tile([128, T], bf16, tag="sqs")
                    for ti in range(2):
                        sl = sqs[:, ti * WSPLIT : (ti + 1) * WSPLIT]
                        if m == 3 and r >= 2:
                            nc.vector._custom_dve(SQ1, out=sl, in0=ps[ti][:])
                        else:
                            nc.scalar.square(sl, ps[ti][:])
                    mag = sq_pool.tile([128, T], bf16, tag="mag")
                    for ti in range(2):
                        sl = slice(ti * WSPLIT, (ti + 1) * WSPLIT)
                        nc.vector._custom_dve(
                            MAG2B, out=mag[:, sl], in0=pc[ti][:],
                            in1=sqs[:, sl],
                        )
                    osb = out_pool.tile([128, T], bf16, tag="osb")
                    nc.scalar.activation(osb[:], mag[:], Sqrt)
                    nc.sync.dma_start(out[r, m, :, :], osb[:])

    nc.compile()
    return nc


def _host_params(win_length, strides, win_pow):
    """Reproduce the reference's parameter transforms on the host."""
    wl = float(np.clip(np.asarray(win_length, np.float64)[0], N / 20.0, float(N)))
    st = float(np.clip(np.asarray(strides, np.float64)[0], 0.0, float(N)))

    es = np.full((T,), st, np.float64)
    frames = np.concatenate([[0.0], np.cumsum(es[1:])])
    idx_floor = np.floor(frames)
    idx_frac = frames - idx_floor

    if not (np.all(idx_frac == 0.0) and np.all(idx_floor == STRIDE * np.arange(T))):
        raise NotImplementedError(
            "kernel fast path requires integer frame stride of 256"
        )

    base = np.arange(N, dtype=np.float64)
    tap = 0.5 - 0.5 * np.cos(2.0 * np.pi * (base + (wl - N + 1) / 2.0) / wl)
    mask = (base >= np.ceil((N - 1 + wl) / 2.0)) | (base <= np.floor((N - 1 - wl) / 2.0))
    tap[mask] = 0.0
    tap = tap / tap.sum()
    tap = tap ** float(np.asarray(win_pow, np.float64)[0])

    if not np.allclose(tap[512:], tap[511::-1], rtol=1e-12, atol=1e-18):
        raise NotImplementedError("kernel fast path requires a symmetric window")
    return tap


def _make_weights(tap):
    """Window-folded half-sample DFT matrices, bf16, [128, NCH, 512]."""
    import ml_dtypes

    wt = tap[512:]  # w~[j] = tap[512+j], j = 0..511
    j = np.arange(512, dtype=np.float64)
    k = np.arange(512, dtype=np.float64)
    ang = 2.0 * np.pi * np.outer(j + 0.5, k) / float(N)
    C = wt[:, None] * np.cos(ang)
    S = wt[:, None] * np.sin(ang)
    C[:, 0] = 0.0                      # slot 0 of c is unused (k=0 on host)
    S[:, 0] = wt * ((-1.0) ** j)       # slot 0 of s is the Nyquist k=512 row

    def pack(M):
        # [512 j, 512 k] -> [128 p, NCH c, 512 k] with j = 128 c + p
        return np.ascontiguousarray(
            M.reshape(NCH, 128, 512).transpose(1, 0, 2).astype(ml_dtypes.bfloat16)
        )

    return pack(C), pack(S)


def _make_in_maps(x, tap):
    import ml_dtypes

    cwb, swb = _make_weights(tap)
    jmb = np.ascontiguousarray(
        np.eye(128, dtype=np.float32)[::-1].astype(ml_dtypes.bfloat16)
    )
    return [
        {
            "xs": np.ascontiguousarray(x[c * BPC : (c + 1) * BPC]),
            "cw": cwb,
            "sw": swb,
            "jm": jmb,
        }
        for c in range(NCORES)
    ]


def kernel(x, win_length, strides, win_pow):
    from concourse.bass_utils import run_bass_kernel_spmd

    x = np.ascontiguousarray(np.asarray(x, dtype=np.float32))
    assert x.shape == (B, L)

    tap = _host_params(win_length, strides, win_pow)

    if "nc" not in _prog_cache:
        _prog_cache["nc"] = _build_program()
    nc = _prog_cache["nc"]

    in_maps = _make_in_maps(x, tap)
    res = run_bass_kernel_spmd(nc, in_maps, core_ids=list(range(NCORES)))
    outp = np.empty((B, F, T), dtype=np.float32)
    for c in range(NCORES):
        o = np.asarray(res.results[c]["out"]).astype(np.float32)
        o = o.reshape(BPC, NMT * 128, T)
        rows = slice(c * BPC, (c + 1) * BPC)
        outp[rows, 1:512] = o[:, 1:512]
        outp[rows, 512] = o[:, 0]       # slot (0,0) carries the Nyquist row

    # k=0 (DC) row on host: X[0] = sum_n w[n] x[., 256t + n]
    wn = tap.astype(np.float32)
    frames_v = np.lib.stride_tricks.as_strided(
        x,
        shape=(B, T, N),
        strides=(x.strides[0], STRIDE * x.itemsize, x.itemsize),
    )
    outp[:, 0, :] = np.abs(frames_v @ wn)
    return outp
